# revision 16
# baseline (speedup 1.0000x reference)
"""Multi-head attention + residual + LayerNorm on 8 Trainium2 NeuronCores.

Problem: nn_MultiHeadAttention_446676599424
  B=2, S=2048, D_MODEL=1024, N_HEAD=16, D_K=64
  reference returns (x, attn_weights):
    x [B, S, D]  = LayerNorm(attn_out @ W_O.T + Q)
    attn_weights [B, H, S, S] = softmax(q k^T / sqrt(d_k))

Sharding: tensor-parallel over heads — 2 heads per core. Each core:
  1. projects q/k/v for its 2 heads (inputs pre-transposed on host so the
     contraction dim lands on SBUF partitions); q/k kept in float32r for
     score precision, v cast to bf16,
  2. computes scores TRANSPOSED  s_T[k, q] = k_proj q_proj^T (so the
     softmax denominator falls out of the attn@v matmul as a ones-column
     of the stationary operand and attn@v needs no transposes),
  3. exp via ScalarE (scale 1/8 fused) into bf16 tiles, denominator =
     row 64 of the context matmul output, replicated across partitions
     with a K=1 PE matmul, reciprocal on DVE,
  4. writes attn in [b, h, k, q] bf16 (host transposes/upcasts),
  5. AllToAll redistributes the per-head context [d, r] to row shards,
  6. every core computes its 512-row shard of context @ W_O.T + residual
     + LayerNorm (fp32) and returns it; host concatenates.
"""

import sys

if "/opt/trn_rl_repo" not in sys.path:
    sys.path.insert(0, "/opt/trn_rl_repo")

import numpy as np

import concourse.bacc as bacc
import concourse.mybir as mybir
from concourse import tile

F32 = mybir.dt.float32
BF16 = mybir.dt.bfloat16
AF = mybir.ActivationFunctionType
ALU = mybir.AluOpType

B = 2
D = 1024
N_HEAD = 16
D_K = 64
N_CORES = 8
H_PER_CORE = N_HEAD // N_CORES  # 2
HD = H_PER_CORE * D_K  # 128, per-core head-dim block
I_TILES = D // 128  # 8 contraction tiles for the projections

# float32r: fp32 storage, full-rate PE with reduced multiply mantissa.
MMDT = mybir.dt.float32r


def build_nc(S=2048):
    """Build the per-core Bass graph (same graph on all 8 cores)."""
    R = B * S  # total rows
    SHARD = R // N_CORES  # output row-shard per core
    KT = S // 128  # k-position tiles per batch
    QW = S // 2  # q-half width (one attention work unit)
    NQC = max(1, QW // 512)
    QC = QW // NQC  # matmul N-chunk
    RC = max(1, R // 512)  # 512-col r-chunks for the projections
    RCW = R // RC
    VT_PER_RC = RCW // 128
    KT_B = KT  # v slots per (head, batch) tile

    nc = bacc.Bacc("TRN2", target_bir_lowering=False, debug=False,
                   num_devices=N_CORES)

    qT = nc.dram_tensor("qT", [D, R], MMDT, kind="ExternalInput")
    kT = nc.dram_tensor("kT", [D, R], MMDT, kind="ExternalInput")
    vT = nc.dram_tensor("vT", [D, R], MMDT, kind="ExternalInput")
    wqT = nc.dram_tensor("wqT", [D, HD], MMDT, kind="ExternalInput")
    wkT = nc.dram_tensor("wkT", [D, HD], MMDT, kind="ExternalInput")
    wvT = nc.dram_tensor("wvT", [D, HD], MMDT, kind="ExternalInput")
    woT = nc.dram_tensor("woT", [D, D], BF16, kind="ExternalInput")
    q_res = nc.dram_tensor("q_res", [SHARD, D], F32, kind="ExternalInput")
    gamma = nc.dram_tensor("gamma", [1, D], MMDT, kind="ExternalInput")
    beta = nc.dram_tensor("beta", [1, D], MMDT, kind="ExternalInput")

    attn_out = nc.dram_tensor("attn_out", [B * H_PER_CORE, S, S], BF16,
                              kind="ExternalOutput")
    x_out = nc.dram_tensor("x_out", [SHARD, D], F32, kind="ExternalOutput")

    with tile.TileContext(nc) as tc:
        with (
            tc.tile_pool(name="persist", bufs=1) as pp,
            tc.tile_pool(name="dram", bufs=1, space="DRAM") as dp,
        ):
            # chunked so phase 2 can start before all projections finish
            qp = [pp.tile([128, RCW], MMDT, tag=f"qp{rc}", name=f"qp{rc}")
                  for rc in range(RC)]
            kp = [pp.tile([128, RCW], MMDT, tag=f"kp{rc}", name=f"kp{rc}")
                  for rc in range(RC)]
            # v slots per (head, batch): [k-tile, 96] bf16
            # (cols 0..63 v, col 64 = 1.0, cols 65..95 pad for 32-mult M)
            v_sb = {(h, b): pp.tile([128, KT_B * 96], BF16,
                                    tag=f"v{h}{b}", name=f"v_sb{h}{b}")
                    for h in range(H_PER_CORE) for b in range(B)}
            # normalized context^T per head [d_k, r], bf16
            ctxn = [pp.tile([64, R], BF16, tag=f"ctxn{h}", name=f"ctxn{h}")
                    for h in range(H_PER_CORE)]
            gamma_rep = pp.tile([128, D], F32, tag="gamma_rep")
            beta_rep = pp.tile([128, D], F32, tag="beta_rep")
            ones_sb = pp.tile([128, 128], MMDT, tag="ones_sb")
            wo_sb = pp.tile([128, I_TILES * D], BF16, tag="wo")
            n_rt = (SHARD + 127) // 128
            qres_sb = pp.tile([128, n_rt * D], F32, tag="qres")

            a2a_in = dp.tile([N_CORES * HD, SHARD], BF16, tag="a2a_in")
            a2a_out = dp.tile([N_CORES * HD, SHARD], BF16, tag="a2a_out")

            # ---- phase 0: constants + weight/residual prefetch ----
            for t in v_sb.values():
                nc.vector.memset(t[:, :], 1.0)
            nc.vector.memset(ones_sb[:, :].bitcast(F32), 1.0)
            gb_sb = pp.tile([1, D], MMDT, tag="gb")
            bb_sb = pp.tile([1, D], MMDT, tag="bb")
            nc.sync.dma_start(out=gb_sb[:, :], in_=gamma[:, :])
            nc.sync.dma_start(out=bb_sb[:, :], in_=beta[:, :])
            for t in range(I_TILES):
                nc.sync.dma_start(out=wo_sb[:, D * t:D * (t + 1)],
                                  in_=woT[128 * t:128 * (t + 1), :])
            for t in range(n_rt):
                rp0 = min(128, SHARD - 128 * t)
                nc.sync.dma_start(
                    out=qres_sb[:rp0, D * t:D * (t + 1)],
                    in_=q_res[128 * t:128 * t + rp0, :])
            with tc.tile_pool(name="ps0", bufs=1, space="PSUM") as ps0:
                for rep, row in ((gamma_rep, gb_sb), (beta_rep, bb_sb)):
                    psb0 = ps0.tile([128, D], F32, tag="psb0",
                                    name=f"psb0_{rep.name}")
                    for jc in range(D // 512):
                        js = slice(512 * jc, 512 * (jc + 1))
                        nc.tensor.matmul(psb0[:, js], ones_sb[0:1, :],
                                         row[0:1, js], start=True, stop=True)
                    nc.vector.tensor_copy(rep[:, :], psb0[:, :])

            # ---- phase 1: projections ----
            with (
                tc.tile_pool(name="p1", bufs=3) as p1,
                tc.tile_pool(name="p1w", bufs=1) as p1w,
                tc.tile_pool(name="ps_qk", bufs=2, space="PSUM") as ps_qk,
                tc.tile_pool(name="ps_v", bufs=4, space="PSUM") as ps_v,
            ):
                wq_sb = p1w.tile([128, D], MMDT, tag="wq")
                wk_sb = p1w.tile([128, D], MMDT, tag="wk")
                wv_sb = p1w.tile([128, D], MMDT, tag="wv")
                for t in range(I_TILES):
                    c = slice(128 * t, 128 * (t + 1))
                    nc.sync.dma_start(out=wq_sb[:, c], in_=wqT[c, :])
                    nc.sync.dma_start(out=wk_sb[:, c], in_=wkT[c, :])
                    nc.sync.dma_start(out=wv_sb[:, c], in_=wvT[c, :])

                GW = min(1024, RCW)  # DMA block r-width
                NG = R // GW
                SUB = GW // RCW if GW > RCW else 1
                for g in range(NG):
                    gs = slice(GW * g, GW * (g + 1))
                    sub_rc = [g * GW // RCW + s for s in range(max(1, GW // RCW))]
                    psq = ps_qk.tile([128, GW], F32, tag="psq",
                                     name=f"psq{g}")
                    psk = ps_qk.tile([128, GW], F32, tag="psk",
                                     name=f"psk{g}")
                    vt_blocks = []
                    for it in range(I_TILES):
                        ic = slice(128 * it, 128 * (it + 1))
                        first, last = it == 0, it == I_TILES - 1
                        qt_b = p1.tile([128, GW], MMDT, tag="qt")
                        kt_b = p1.tile([128, GW], MMDT, tag="kt")
                        vt_b = p1.tile([128, GW], MMDT, tag="vt",
                                       bufs=I_TILES + 1)
                        nc.sync.dma_start(out=qt_b[:, :], in_=qT[ic, gs])
                        nc.sync.dma_start(out=kt_b[:, :], in_=kT[ic, gs])
                        nc.sync.dma_start(out=vt_b[:, :], in_=vT[ic, gs])
                        vt_blocks.append(vt_b)
                        for s in range(len(sub_rc)):
                            ss_ = slice(RCW * s, RCW * (s + 1)) \
                                if GW > RCW else slice(0, GW)
                            nc.tensor.matmul(psq[:, ss_], wq_sb[:, ic],
                                             qt_b[:, ss_],
                                             start=first, stop=last)
                            nc.tensor.matmul(psk[:, ss_], wk_sb[:, ic],
                                             kt_b[:, ss_],
                                             start=first, stop=last)
                    for s, rc in enumerate(sub_rc):
                        ss_ = slice(RCW * s, RCW * (s + 1)) \
                            if GW > RCW else slice(0, GW)
                        nc.vector.tensor_copy(qp[rc][:, :], psq[:, ss_])
                        nc.vector.tensor_copy(kp[rc][:, :], psk[:, ss_])
                    for t in range(GW // 128):
                        psv = ps_v.tile([128, 128], F32, tag="psv",
                                        name=f"psv{g}_{t}")
                        for it in range(I_TILES):
                            ic = slice(128 * it, 128 * (it + 1))
                            nc.tensor.matmul(
                                psv[:, :],
                                vt_blocks[it][:, 128 * t:128 * (t + 1)],
                                wv_sb[:, ic],
                                start=(it == 0), stop=(it == I_TILES - 1))
                        r_tile = g * (GW // 128) + t  # global r tile
                        b_ix, kt_ix = divmod(r_tile, KT_B)
                        for h in range(H_PER_CORE):
                            nc.vector.tensor_copy(
                                v_sb[(h, b_ix)][:, 96 * kt_ix:96 * kt_ix + 64],
                                psv[:, 64 * h:64 * (h + 1)])

            # ---- phase 2: attention ----
            def pkslice(col0, width):
                """(tile_index, slice) within the chunked qp/kp tiles."""
                t = col0 // RCW
                o = col0 - t * RCW
                assert o + width <= RCW
                return t, slice(o, o + width)

            with (
                tc.tile_pool(name="p2a", bufs=2) as p2a,
                tc.tile_pool(name="p2w", bufs=4) as p2w,
                tc.tile_pool(name="p2r", bufs=2) as p2r,
                tc.tile_pool(name="ps_s", bufs=2, space="PSUM") as ps_s,
                tc.tile_pool(name="ps_c", bufs=1, space="PSUM") as ps_c,
            ):
                for b in range(B):
                    for h in range(H_PER_CORE):
                        hs = slice(64 * h, 64 * (h + 1))
                        for qh in range(2):
                            q0 = S * b + QW * qh
                            at_tiles = []
                            psc = ps_c.tile([128, QW], F32, tag="psc")

                            def scores(kt_i, _q0=q0, _hs=hs):
                                kt_t, kt_s = pkslice(S * b + 128 * kt_i, 128)
                                ps = ps_s.tile([128, QW], F32, tag="pss",
                                               name=f"pss{b}{h}{qh}_{kt_i}")
                                for qc in range(NQC):
                                    qt_t, qt_s = pkslice(_q0 + QC * qc, QC)
                                    nc.tensor.matmul(
                                        ps[:, QC * qc:QC * (qc + 1)],
                                        kp[kt_t][_hs, kt_s],
                                        qp[qt_t][_hs, qt_s],
                                        start=True, stop=True)
                                return ps

                            ps_prev = scores(0)
                            for kt_i in range(KT):
                                at = p2a.tile([128, QW], BF16,
                                              tag=f"attn{kt_i}")
                                nc.scalar.activation(at[:, :], ps_prev[:, :],
                                                     AF.Exp, scale=0.125)
                                at_tiles.append(at)
                                if kt_i + 1 < KT:
                                    ps_prev = scores(kt_i + 1)
                                for qc in range(NQC):
                                    cs = slice(QC * qc, QC * (qc + 1))
                                    nc.tensor.matmul(
                                        psc[0:96, cs],
                                        v_sb[(h, b)][:, 96 * kt_i:
                                                     96 * kt_i + 96],
                                        at[:, cs],
                                        start=(kt_i == 0),
                                        stop=(kt_i == KT - 1))
                            # denominator -> replicated reciprocal
                            den_sb = p2r.tile([128, QW], MMDT, tag="den_sb")
                            recip = p2r.tile([128, QW], F32, tag="recip")
                            recip_b = p2r.tile([128, QW], BF16, tag="recip_b")
                            psb = ps_s.tile([128, QW], F32, tag="pss",
                                            name=f"psb{b}{h}{qh}")
                            nc.vector.tensor_copy(den_sb[64:65, :],
                                                  psc[64:65, :])
                            for qc in range(NQC):
                                cs = slice(QC * qc, QC * (qc + 1))
                                nc.tensor.matmul(psb[:, cs],
                                                 ones_sb[64:65, :],
                                                 den_sb[64:65, cs],
                                                 start=True, stop=True)
                            nc.vector.reciprocal_approx_fast(recip[:, :],
                                                             psb[:, :])
                            nc.vector.tensor_copy(recip_b[:, :], recip[:, :])
                            # normalized context^T for this unit
                            nc.vector.tensor_mul(
                                ctxn[h][0:64, q0:q0 + QW],
                                psc[0:64, :], recip[0:64, :])
                            # normalize + write attention tiles
                            plane = b * H_PER_CORE + h
                            for kt_i in range(KT):
                                wt = p2w.tile([128, QW], BF16, tag="wt")
                                nc.vector.tensor_mul(wt[:, :],
                                                     at_tiles[kt_i][:, :],
                                                     recip_b[:, :])
                                nc.gpsimd.dma_start(
                                    out=attn_out[plane,
                                                 128 * kt_i:128 * (kt_i + 1),
                                                 QW * qh:QW * (qh + 1)],
                                    in_=wt[:, :])

            # ---- phase 3: all-to-all of the context ----
            for j in range(N_CORES):
                ss = slice(SHARD * j, SHARD * (j + 1))
                for h in range(H_PER_CORE):
                    nc.sync.dma_start(
                        out=a2a_in[HD * j + 64 * h:HD * j + 64 * (h + 1), :],
                        in_=ctxn[h][0:64, ss])
            nc.gpsimd.collective_compute(
                "AllToAll",
                ALU.bypass,
                replica_groups=[list(range(N_CORES))],
                ins=[a2a_in[:, :].opt()],
                outs=[a2a_out[:, :].opt()],
            )

            # ---- phase 4: W_O projection + residual + LayerNorm ----
            with (
                tc.tile_pool(name="p4", bufs=1) as p4,
                tc.tile_pool(name="p4x", bufs=2) as p4x,
                tc.tile_pool(name="ps_o", bufs=2, space="PSUM") as ps_o,
            ):
                ctxf = p4.tile([128, I_TILES * SHARD], BF16, tag="ctxf")
                for t in range(I_TILES):
                    nc.sync.dma_start(
                        out=ctxf[:, SHARD * t:SHARD * (t + 1)],
                        in_=a2a_out[128 * t:128 * (t + 1), :])

                for rt in range(n_rt):
                    rp = min(128, SHARD - 128 * rt)
                    rsl = slice(128 * rt, 128 * rt + rp)
                    xsb = p4x.tile([128, D], F32, tag="xsb")
                    for jc in range(2):
                        js = slice(512 * jc, 512 * (jc + 1))
                        pso = ps_o.tile([128, 512], F32, tag="pso")
                        for t in range(I_TILES):
                            nc.tensor.matmul(
                                pso[:rp, :],
                                ctxf[:, SHARD * t + 128 * rt:
                                     SHARD * t + 128 * rt + rp],
                                wo_sb[:, D * t + 512 * jc:
                                      D * t + 512 * (jc + 1)],
                                start=(t == 0), stop=(t == I_TILES - 1))
                        nc.vector.tensor_add(
                            xsb[:rp, js], pso[:rp, :],
                            qres_sb[:rp, D * rt + 512 * jc:
                                    D * rt + 512 * (jc + 1)])
                    # LayerNorm over the free dim (D)
                    mu = p4x.tile([128, 1], F32, tag="mu")
                    var = p4x.tile([128, 1], F32, tag="var")
                    rstd = p4x.tile([128, 1], F32, tag="rstd")
                    xc = p4x.tile([128, D], F32, tag="xc")
                    sq = p4x.tile([128, D], F32, tag="sq")
                    xo = p4x.tile([128, D], F32, tag="xo")
                    nc.vector.tensor_reduce(mu[:rp, :], xsb[:rp, :],
                                            axis=mybir.AxisListType.X,
                                            op=ALU.add)
                    nc.vector.tensor_scalar_mul(mu[:rp, :], mu[:rp, :],
                                                1.0 / D)
                    nc.vector.tensor_scalar(xc[:rp, :], xsb[:rp, :],
                                            mu[:rp, :], None,
                                            op0=ALU.subtract)
                    nc.vector.scalar_tensor_tensor(
                        sq[:rp, :], in0=xc[:rp, :], scalar=1.0,
                        in1=xc[:rp, :], op0=ALU.mult, op1=ALU.mult,
                        accum_out=var[:rp, :])
                    nc.vector.tensor_scalar(var[:rp, :], var[:rp, :],
                                            1.0 / D, 1e-5,
                                            op0=ALU.mult, op1=ALU.add)
                    # rstd = exp(-0.5 * ln(var+eps)) (Ln/Exp share a table)
                    nc.scalar.activation(rstd[:rp, :], var[:rp, :], AF.Ln)
                    nc.scalar.activation(rstd[:rp, :], rstd[:rp, :], AF.Exp,
                                         scale=-0.5)
                    nc.vector.scalar_tensor_tensor(
                        xo[:rp, :], in0=xc[:rp, :], scalar=rstd[:rp, :],
                        in1=gamma_rep[:rp, :], op0=ALU.mult, op1=ALU.mult)
                    nc.vector.tensor_add(xo[:rp, :], xo[:rp, :],
                                         beta_rep[:rp, :])
                    nc.sync.dma_start(out=x_out[rsl, :], in_=xo[:rp, :])

    nc.compile()
    return nc


_NC_CACHE = {}


def _get_nc(S):
    if S not in _NC_CACHE:
        _NC_CACHE[S] = build_nc(S)
    return _NC_CACHE[S]


def make_in_maps(Q, K, V, W_Q, W_K, W_V, W_O, ln_gamma, ln_beta):
    S = Q.shape[1]
    R = B * S
    SHARD = R // N_CORES
    bf16 = mybir.dt.np(BF16)
    Q2 = np.asarray(Q, np.float32).reshape(R, D)
    QT = np.ascontiguousarray(Q2.T)
    KT_ = np.ascontiguousarray(np.asarray(K, np.float32).reshape(R, D).T)
    VT_ = np.ascontiguousarray(np.asarray(V, np.float32).reshape(R, D).T)
    woT = np.ascontiguousarray(np.asarray(W_O, np.float32).T).astype(bf16)
    g = np.asarray(ln_gamma, np.float32).reshape(1, D)
    bta = np.asarray(ln_beta, np.float32).reshape(1, D)
    in_maps = []
    for c in range(N_CORES):
        hsl = slice(HD * c, HD * (c + 1))
        in_maps.append({
            "qT": QT, "kT": KT_, "vT": VT_,
            "wqT": np.ascontiguousarray(np.asarray(W_Q, np.float32)[hsl, :].T),
            "wkT": np.ascontiguousarray(np.asarray(W_K, np.float32)[hsl, :].T),
            "wvT": np.ascontiguousarray(np.asarray(W_V, np.float32)[hsl, :].T),
            "woT": woT,
            "q_res": np.ascontiguousarray(Q2[SHARD * c:SHARD * (c + 1), :]),
            "gamma": g, "beta": bta,
        })
    return in_maps


def assemble(results, S):
    R = B * S
    SHARD = R // N_CORES
    x = np.empty((R, D), np.float32)
    attn = np.empty((B, N_HEAD, S, S), np.float32)
    for c in range(N_CORES):
        x[SHARD * c:SHARD * (c + 1), :] = results[c]["x_out"]
        a = np.asarray(results[c]["attn_out"], dtype=np.float32)
        for b in range(B):
            for h in range(H_PER_CORE):
                attn[b, H_PER_CORE * c + h] = a[b * H_PER_CORE + h].T
    return x.reshape(B, S, D), attn


def kernel(Q, K, V, W_Q, W_K, W_V, W_O, ln_gamma, ln_beta):
    from concourse.bass_utils import run_bass_kernel_spmd

    S = Q.shape[1]
    nc = _get_nc(S)
    in_maps = make_in_maps(Q, K, V, W_Q, W_K, W_V, W_O, ln_gamma, ln_beta)
    res = run_bass_kernel_spmd(nc, in_maps, core_ids=list(range(N_CORES)))
    return assemble(res.results, S)


# revision 22
# speedup vs baseline: 1.2674x; 1.2674x over previous
"""Multi-head attention + residual + LayerNorm on 8 Trainium2 NeuronCores.

Problem: nn_MultiHeadAttention_446676599424
  B=2, S=2048, D_MODEL=1024, N_HEAD=16, D_K=64
  reference returns (x, attn_weights):
    x [B, S, D]  = LayerNorm(attn_out @ W_O.T + Q)
    attn_weights [B, H, S, S] = softmax(q k^T / sqrt(d_k))

Sharding: tensor-parallel over heads — 2 heads per core. Each core:
  1. projects q/k/v for its 2 heads (inputs pre-transposed on host so the
     contraction dim lands on SBUF partitions); q/k kept in float32r for
     score precision, v cast to bf16,
  2. computes scores TRANSPOSED  s_T[k, q] = k_proj q_proj^T (so the
     softmax denominator falls out of the attn@v matmul as a ones-column
     of the stationary operand and attn@v needs no transposes),
  3. exp via ScalarE (scale 1/8 fused) into bf16 tiles, denominator =
     row 64 of the context matmul output, replicated across partitions
     with a K=1 PE matmul, reciprocal on DVE,
  4. writes attn in [b, h, k, q] bf16 (host transposes/upcasts),
  5. AllToAll redistributes the per-head context [d, r] to row shards,
  6. every core computes its 512-row shard of context @ W_O.T + residual
     + LayerNorm (fp32) and returns it; host concatenates.
"""

import sys

if "/opt/trn_rl_repo" not in sys.path:
    sys.path.insert(0, "/opt/trn_rl_repo")

import numpy as np

import concourse.bacc as bacc
import concourse.mybir as mybir
from concourse import tile

F32 = mybir.dt.float32
BF16 = mybir.dt.bfloat16
AF = mybir.ActivationFunctionType
ALU = mybir.AluOpType

B = 2
D = 1024
N_HEAD = 16
D_K = 64
N_CORES = 8
H_PER_CORE = N_HEAD // N_CORES  # 2
HD = H_PER_CORE * D_K  # 128, per-core head-dim block
I_TILES = D // 128  # 8 contraction tiles for the projections

# float32r: fp32 storage, full-rate PE with reduced multiply mantissa.
MMDT = mybir.dt.float32r


def build_nc(S=2048):
    """Build the per-core Bass graph (same graph on all 8 cores)."""
    R = B * S  # total rows
    SHARD = R // N_CORES  # output row-shard per core
    KT = S // 128  # k-position tiles per batch
    QW = S // 2  # q-half width (one attention work unit)
    NQC = max(1, QW // 512)
    QC = QW // NQC  # matmul N-chunk
    RCW = min(512, S)  # qp/kp chunk width
    RC = R // RCW
    RC_B = RC // B  # chunks per batch
    KT_B = KT

    nc = bacc.Bacc("TRN2", target_bir_lowering=False, debug=False,
                   num_devices=N_CORES)

    qT = nc.dram_tensor("qT", [D, R], BF16, kind="ExternalInput")
    kT = nc.dram_tensor("kT", [D, R], BF16, kind="ExternalInput")
    vT = nc.dram_tensor("vT", [D, R], BF16, kind="ExternalInput")
    wqT = nc.dram_tensor("wqT", [D, HD], BF16, kind="ExternalInput")
    wkT = nc.dram_tensor("wkT", [D, HD], BF16, kind="ExternalInput")
    wvT = nc.dram_tensor("wvT", [D, HD], BF16, kind="ExternalInput")
    woT = nc.dram_tensor("woT", [D, D], BF16, kind="ExternalInput")
    q_res = nc.dram_tensor("q_res", [SHARD, D], F32, kind="ExternalInput")
    gamma = nc.dram_tensor("gamma", [1, D], MMDT, kind="ExternalInput")
    beta = nc.dram_tensor("beta", [1, D], MMDT, kind="ExternalInput")

    attn_out = nc.dram_tensor("attn_out", [B * H_PER_CORE, S, S], BF16,
                              kind="ExternalOutput")
    x_out = nc.dram_tensor("x_out", [SHARD, D], F32, kind="ExternalOutput")

    from contextlib import ExitStack

    with tile.TileContext(nc) as tc:
        with (
            tc.tile_pool(name="persist", bufs=1) as pp,
            tc.tile_pool(name="dram", bufs=1, space="DRAM") as dp,
        ):
            qp = [pp.tile([128, RCW], BF16, tag=f"qp{rc}", name=f"qp{rc}")
                  for rc in range(RC)]
            kp = [pp.tile([128, RCW], BF16, tag=f"kp{rc}", name=f"kp{rc}")
                  for rc in range(RC)]
            v_sb = {(h, b): pp.tile([128, KT_B * 96], BF16,
                                    tag=f"v{h}{b}", name=f"v_sb{h}{b}")
                    for h in range(H_PER_CORE) for b in range(B)}
            ctxn = [pp.tile([64, R], BF16, tag=f"ctxn{h}", name=f"ctxn{h}")
                    for h in range(H_PER_CORE)]
            gamma_rep = pp.tile([128, D], F32, tag="gamma_rep")
            beta_rep = pp.tile([128, D], F32, tag="beta_rep")
            ones_sb = pp.tile([128, 128], MMDT, tag="ones_sb")
            n_rt = (SHARD + 127) // 128
            wq_sb = pp.tile([128, D], BF16, tag="wq")
            wk_sb = pp.tile([128, D], BF16, tag="wk")
            wv_sb = pp.tile([128, D], BF16, tag="wv")

            a2a_in = dp.tile([N_CORES * HD, SHARD], BF16, tag="a2a_in")
            a2a_out = dp.tile([N_CORES * HD, SHARD], BF16, tag="a2a_out")

            # ---- phase 0: constants + weight/residual prefetch ----
            for t in v_sb.values():
                nc.vector.memset(t[:, :], 1.0)
            nc.vector.memset(ones_sb[:, :].bitcast(F32), 1.0)
            gb_sb = pp.tile([1, D], MMDT, tag="gb")
            bb_sb = pp.tile([1, D], MMDT, tag="bb")
            nc.sync.dma_start(out=gb_sb[:, :], in_=gamma[:, :])
            nc.sync.dma_start(out=bb_sb[:, :], in_=beta[:, :])
            for wsb, wdr in ((wq_sb, wqT), (wk_sb, wkT), (wv_sb, wvT)):
                nc.sync.dma_start(
                    out=wsb[:, :].rearrange("p (t m) -> p t m", t=I_TILES),
                    in_=wdr[:, :].rearrange("(t p) m -> p t m", p=128))
            stk = ExitStack()
            p1 = stk.enter_context(tc.tile_pool(name="p1", bufs=1))
            p2a = stk.enter_context(tc.tile_pool(name="p2a", bufs=2))
            p2w = stk.enter_context(tc.tile_pool(name="p2w", bufs=3))
            p2r = stk.enter_context(tc.tile_pool(name="p2r", bufs=2))
            ps_1 = stk.enter_context(
                tc.tile_pool(name="ps_1", bufs=1, space="PSUM"))
            ps_v = stk.enter_context(
                tc.tile_pool(name="ps_v", bufs=2, space="PSUM"))
            ps_s = stk.enter_context(
                tc.tile_pool(name="ps_s", bufs=2, space="PSUM"))
            ps_c = stk.enter_context(
                tc.tile_pool(name="ps_c", bufs=1, space="PSUM"))
            if True:
                for rep, row in ((gamma_rep, gb_sb), (beta_rep, bb_sb)):
                    psb0 = ps_c.tile([128, D], F32, tag="psc",
                                     name=f"psb0_{rep.name}")
                    for jc in range(D // 512):
                        js = slice(512 * jc, 512 * (jc + 1))
                        nc.tensor.matmul(psb0[:, js], ones_sb[0:1, :],
                                         row[0:1, js], start=True, stop=True)
                    nc.vector.tensor_copy(rep[:, :], psb0[:, :])

            def phase1(b):
                """Project q/k/v for batch b's rows."""
                bufs = 1
                for rcl in range(RC_B):
                    rc = b * RC_B + rcl
                    rs = slice(RCW * rc, RCW * (rc + 1))
                    qt_b = p1.tile([128, I_TILES * RCW], BF16,
                                   tag=f"qt{b}", bufs=bufs,
                                   name=f"qt{b}_{rcl}")
                    kt_b = p1.tile([128, I_TILES * RCW], BF16,
                                   tag=f"kt{b}", bufs=bufs,
                                   name=f"kt{b}_{rcl}")
                    vt_b = p1.tile([128, I_TILES * RCW], BF16,
                                   tag=f"vt{b}", bufs=bufs,
                                   name=f"vt{b}_{rcl}")
                    for sb_t, dr_t in ((qt_b, qT), (kt_b, kT),
                                       (vt_b, vT)):
                        nc.sync.dma_start(
                            out=sb_t[:, :].rearrange("p (t r) -> p t r",
                                                     t=I_TILES),
                            in_=dr_t[:, rs].rearrange("(t p) r -> p t r",
                                                      p=128))
                    psq = ps_1.tile([128, RCW], F32, tag="psq",
                                    name=f"psq{rc}")
                    psk = ps_1.tile([128, RCW], F32, tag="psk",
                                    name=f"psk{rc}")
                    for it in range(I_TILES):
                        ic = slice(128 * it, 128 * (it + 1))
                        ir = slice(RCW * it, RCW * (it + 1))
                        first, last = it == 0, it == I_TILES - 1
                        nc.tensor.matmul(psq[:, :], wq_sb[:, ic],
                                         qt_b[:, ir], start=first, stop=last)
                        nc.tensor.matmul(psk[:, :], wk_sb[:, ic],
                                         kt_b[:, ir], start=first, stop=last)
                    nc.vector.tensor_copy(qp[rc][:, :], psq[:, :])
                    nc.vector.tensor_copy(kp[rc][:, :], psk[:, :])
                    for t in range(RCW // 128):
                        psv = ps_v.tile([128, 128], F32, tag="psv",
                                        name=f"psv{rc}_{t}")
                        for it in range(I_TILES):
                            ic = slice(128 * it, 128 * (it + 1))
                            nc.tensor.matmul(
                                psv[:, :],
                                vt_b[:, RCW * it + 128 * t:
                                     RCW * it + 128 * (t + 1)],
                                wv_sb[:, ic],
                                start=(it == 0), stop=(it == I_TILES - 1))
                        r_tile = rc * (RCW // 128) + t
                        b_ix, kt_ix = divmod(r_tile, KT_B)
                        for h in range(H_PER_CORE):
                            nc.vector.tensor_copy(
                                v_sb[(h, b_ix)][:, 96 * kt_ix:96 * kt_ix + 64],
                                psv[:, 64 * h:64 * (h + 1)])

            def pkslice(col0, width):
                t = col0 // RCW
                o = col0 - t * RCW
                assert o + width <= RCW
                return t, slice(o, o + width)

            def phase2(b):
                """Attention for batch b."""
                for h in range(H_PER_CORE):
                    hs = slice(64 * h, 64 * (h + 1))
                    for qh in range(2):
                        q0 = S * b + QW * qh
                        chunks = [(kt_i, qc) for kt_i in range(KT)
                                  for qc in range(NQC)]
                        psc = ps_c.tile([128, QW], F32, tag="psc",
                                        name=f"psc{b}{h}{qh}")
                        pss_of = {}

                        def scores(i, _q0=q0, _hs=hs, _b=b, _h=h, _qh=qh):
                            kt_i, qc = chunks[i]
                            kt_t, kt_s = pkslice(S * _b + 128 * kt_i, 128)
                            ps = ps_s.tile([128, QC], F32, tag="pss",
                                           name=f"pss{_b}{_h}{_qh}_{i}")
                            qt_t, qt_s = pkslice(_q0 + QC * qc, QC)
                            nc.tensor.matmul(ps[:, :], kp[kt_t][_hs, kt_s],
                                             qp[qt_t][_hs, qt_s],
                                             start=True, stop=True)
                            pss_of[i] = ps

                        scores(0)
                        at_tiles = []
                        for i, (kt_i, qc) in enumerate(chunks):
                            if qc == 0:
                                at = p2a.tile([128, QW], BF16,
                                              tag=f"attn{kt_i}",
                                              name=f"at{b}{h}{qh}_{kt_i}")
                                at_tiles.append(at)
                            cs = slice(QC * qc, QC * (qc + 1))
                            nc.scalar.activation(at_tiles[kt_i][:, cs],
                                                 pss_of.pop(i)[:, :],
                                                 AF.Exp, scale=0.125)
                            if i + 1 < len(chunks):
                                scores(i + 1)
                            nc.tensor.matmul(
                                psc[0:96, cs],
                                v_sb[(h, b)][:, 96 * kt_i:96 * kt_i + 96],
                                at_tiles[kt_i][:, cs],
                                start=(kt_i == 0), stop=(kt_i == KT - 1))
                        # denominator -> replicated reciprocal
                        den_sb = p2r.tile([128, QW], MMDT, tag="den_sb",
                                          bufs=1)
                        recip = p2r.tile([128, QW], F32, tag="recip")
                        recip_b = p2r.tile([128, QW], BF16, tag="recip_b")
                        nc.vector.tensor_copy(den_sb[64:65, :],
                                              psc[64:65, :])
                        for qc in range(NQC):
                            cs = slice(QC * qc, QC * (qc + 1))
                            psb = ps_s.tile([128, QC], F32, tag="pss",
                                            name=f"psb{b}{h}{qh}_{qc}")
                            nc.tensor.matmul(psb[:, :], ones_sb[64:65, :],
                                             den_sb[64:65, cs],
                                             start=True, stop=True)
                            nc.vector.reciprocal_approx_fast(recip[:, cs],
                                                             psb[:, :])
                        nc.vector.tensor_copy(recip_b[:, :], recip[:, :])
                        nc.vector.tensor_mul(
                            ctxn[h][0:64, q0:q0 + QW],
                            psc[0:64, :], recip[0:64, :])
                        plane = b * H_PER_CORE + h
                        for kt_i in range(KT):
                            wt = p2w.tile([128, QW], BF16, tag="wt")
                            nc.vector.tensor_mul(wt[:, :],
                                                 at_tiles[kt_i][:, :],
                                                 recip_b[:, :])
                            nc.gpsimd.dma_start(
                                out=attn_out[plane,
                                             128 * kt_i:128 * (kt_i + 1),
                                             QW * qh:QW * (qh + 1)],
                                in_=wt[:, :])

            for b in range(B):
                phase1(b)
                phase2(b)
            stk.close()

            # ---- phase 3: all-to-all of the context ----
            for j in range(N_CORES):
                ss = slice(SHARD * j, SHARD * (j + 1))
                for h in range(H_PER_CORE):
                    nc.sync.dma_start(
                        out=a2a_in[HD * j + 64 * h:HD * j + 64 * (h + 1), :],
                        in_=ctxn[h][0:64, ss])
            nc.gpsimd.collective_compute(
                "AllToAll",
                ALU.bypass,
                replica_groups=[list(range(N_CORES))],
                ins=[a2a_in[:, :].opt()],
                outs=[a2a_out[:, :].opt()],
            )

            # ---- phase 4: W_O projection + residual + LayerNorm ----
            with (
                tc.tile_pool(name="p4", bufs=1) as p4,
                tc.tile_pool(name="p4x", bufs=2) as p4x,
                tc.tile_pool(name="ps_o", bufs=2, space="PSUM") as ps_o,
            ):
                wo_sb = p4.tile([128, I_TILES * D], BF16, tag="wo")
                qres_sb = p4.tile([128, n_rt * D], F32, tag="qres")
                for t in range(I_TILES):
                    nc.sync.dma_start(out=wo_sb[:, D * t:D * (t + 1)],
                                      in_=woT[128 * t:128 * (t + 1), :])
                for t in range(n_rt):
                    rp0 = min(128, SHARD - 128 * t)
                    nc.sync.dma_start(
                        out=qres_sb[:rp0, D * t:D * (t + 1)],
                        in_=q_res[128 * t:128 * t + rp0, :])
                ctxf = p4.tile([128, I_TILES * SHARD], BF16, tag="ctxf")
                for t in range(I_TILES):
                    nc.sync.dma_start(
                        out=ctxf[:, SHARD * t:SHARD * (t + 1)],
                        in_=a2a_out[128 * t:128 * (t + 1), :])

                for rt in range(n_rt):
                    rp = min(128, SHARD - 128 * rt)
                    rsl = slice(128 * rt, 128 * rt + rp)
                    xsb = p4x.tile([128, D], F32, tag="xsb")
                    for jc in range(2):
                        js = slice(512 * jc, 512 * (jc + 1))
                        pso = ps_o.tile([128, 512], F32, tag="pso")
                        for t in range(I_TILES):
                            nc.tensor.matmul(
                                pso[:rp, :],
                                ctxf[:, SHARD * t + 128 * rt:
                                     SHARD * t + 128 * rt + rp],
                                wo_sb[:, D * t + 512 * jc:
                                      D * t + 512 * (jc + 1)],
                                start=(t == 0), stop=(t == I_TILES - 1))
                        nc.vector.tensor_add(
                            xsb[:rp, js], pso[:rp, :],
                            qres_sb[:rp, D * rt + 512 * jc:
                                    D * rt + 512 * (jc + 1)])
                    mu = p4x.tile([128, 1], F32, tag="mu")
                    var = p4x.tile([128, 1], F32, tag="var")
                    rstd = p4x.tile([128, 1], F32, tag="rstd")
                    xc = p4x.tile([128, D], F32, tag="xc")
                    sq = p4x.tile([128, D], F32, tag="sq")
                    xo = p4x.tile([128, D], F32, tag="xo")
                    nc.vector.tensor_reduce(mu[:rp, :], xsb[:rp, :],
                                            axis=mybir.AxisListType.X,
                                            op=ALU.add)
                    nc.vector.tensor_scalar_mul(mu[:rp, :], mu[:rp, :],
                                                1.0 / D)
                    nc.vector.tensor_scalar(xc[:rp, :], xsb[:rp, :],
                                            mu[:rp, :], None,
                                            op0=ALU.subtract)
                    nc.vector.scalar_tensor_tensor(
                        sq[:rp, :], in0=xc[:rp, :], scalar=1.0,
                        in1=xc[:rp, :], op0=ALU.mult, op1=ALU.mult,
                        accum_out=var[:rp, :])
                    nc.vector.tensor_scalar(var[:rp, :], var[:rp, :],
                                            1.0 / D, 1e-5,
                                            op0=ALU.mult, op1=ALU.add)
                    nc.scalar.activation(rstd[:rp, :], var[:rp, :], AF.Ln)
                    nc.scalar.activation(rstd[:rp, :], rstd[:rp, :], AF.Exp,
                                         scale=-0.5)
                    nc.vector.scalar_tensor_tensor(
                        xo[:rp, :], in0=xc[:rp, :], scalar=rstd[:rp, :],
                        in1=gamma_rep[:rp, :], op0=ALU.mult, op1=ALU.mult)
                    nc.vector.tensor_add(xo[:rp, :], xo[:rp, :],
                                         beta_rep[:rp, :])
                    nc.sync.dma_start(out=x_out[rsl, :], in_=xo[:rp, :])

    nc.compile()
    return nc


_NC_CACHE = {}


def _get_nc(S):
    if S not in _NC_CACHE:
        _NC_CACHE[S] = build_nc(S)
    return _NC_CACHE[S]


def make_in_maps(Q, K, V, W_Q, W_K, W_V, W_O, ln_gamma, ln_beta):
    S = Q.shape[1]
    R = B * S
    SHARD = R // N_CORES
    bf16 = mybir.dt.np(BF16)
    Q2 = np.asarray(Q, np.float32).reshape(R, D)
    QT = np.ascontiguousarray(Q2.T)
    KT_ = np.ascontiguousarray(np.asarray(K, np.float32).reshape(R, D).T)
    VT_ = np.ascontiguousarray(np.asarray(V, np.float32).reshape(R, D).T)
    woT = np.ascontiguousarray(np.asarray(W_O, np.float32).T).astype(bf16)
    QTb = QT.astype(bf16)
    KTb = KT_.astype(bf16)
    VTb = VT_.astype(bf16)
    g = np.asarray(ln_gamma, np.float32).reshape(1, D)
    bta = np.asarray(ln_beta, np.float32).reshape(1, D)
    in_maps = []
    for c in range(N_CORES):
        hsl = slice(HD * c, HD * (c + 1))
        in_maps.append({
            "qT": QTb, "kT": KTb, "vT": VTb,
            "wqT": np.ascontiguousarray(
                np.asarray(W_Q, np.float32)[hsl, :].T).astype(bf16),
            "wkT": np.ascontiguousarray(
                np.asarray(W_K, np.float32)[hsl, :].T).astype(bf16),
            "wvT": np.ascontiguousarray(
                np.asarray(W_V, np.float32)[hsl, :].T).astype(bf16),
            "woT": woT,
            "q_res": np.ascontiguousarray(Q2[SHARD * c:SHARD * (c + 1), :]),
            "gamma": g, "beta": bta,
        })
    return in_maps


def assemble(results, S):
    R = B * S
    SHARD = R // N_CORES
    x = np.empty((R, D), np.float32)
    attn = np.empty((B, N_HEAD, S, S), np.float32)
    for c in range(N_CORES):
        x[SHARD * c:SHARD * (c + 1), :] = results[c]["x_out"]
        a = np.asarray(results[c]["attn_out"], dtype=np.float32)
        for b in range(B):
            for h in range(H_PER_CORE):
                attn[b, H_PER_CORE * c + h] = a[b * H_PER_CORE + h].T
    return x.reshape(B, S, D), attn


def kernel(Q, K, V, W_Q, W_K, W_V, W_O, ln_gamma, ln_beta):
    from concourse.bass_utils import run_bass_kernel_spmd

    S = Q.shape[1]
    nc = _get_nc(S)
    in_maps = make_in_maps(Q, K, V, W_Q, W_K, W_V, W_O, ln_gamma, ln_beta)
    res = run_bass_kernel_spmd(nc, in_maps, core_ids=list(range(N_CORES)))
    return assemble(res.results, S)


# revision 23
# speedup vs baseline: 1.5020x; 1.1851x over previous
"""Multi-head attention + residual + LayerNorm on 8 Trainium2 NeuronCores.

Problem: nn_MultiHeadAttention_446676599424
  B=2, S=2048, D_MODEL=1024, N_HEAD=16, D_K=64
  reference returns (x, attn_weights):
    x [B, S, D]  = LayerNorm(attn_out @ W_O.T + Q)
    attn_weights [B, H, S, S] = softmax(q k^T / sqrt(d_k))

Sharding: tensor-parallel over heads — 2 heads per core. Each core:
  1. projects q/k/v for its 2 heads (inputs pre-transposed on host so the
     contraction dim lands on SBUF partitions); q/k kept in float32r for
     score precision, v cast to bf16,
  2. computes scores TRANSPOSED  s_T[k, q] = k_proj q_proj^T (so the
     softmax denominator falls out of the attn@v matmul as a ones-column
     of the stationary operand and attn@v needs no transposes),
  3. exp via ScalarE (scale 1/8 fused) into bf16 tiles, denominator =
     row 64 of the context matmul output, replicated across partitions
     with a K=1 PE matmul, reciprocal on DVE,
  4. writes attn in [b, h, k, q] bf16 (host transposes/upcasts),
  5. AllToAll redistributes the per-head context [d, r] to row shards,
  6. every core computes its 512-row shard of context @ W_O.T + residual
     + LayerNorm (fp32) and returns it; host concatenates.
"""

import sys

if "/opt/trn_rl_repo" not in sys.path:
    sys.path.insert(0, "/opt/trn_rl_repo")

import numpy as np

import concourse.bacc as bacc
import concourse.mybir as mybir
from concourse import tile

F32 = mybir.dt.float32
BF16 = mybir.dt.bfloat16
AF = mybir.ActivationFunctionType
ALU = mybir.AluOpType

B = 2
D = 1024
N_HEAD = 16
D_K = 64
N_CORES = 8
H_PER_CORE = N_HEAD // N_CORES  # 2
HD = H_PER_CORE * D_K  # 128, per-core head-dim block
I_TILES = D // 128  # 8 contraction tiles for the projections

# float32r: fp32 storage, full-rate PE with reduced multiply mantissa.
MMDT = mybir.dt.float32r


def build_nc(S=2048):
    """Build the per-core Bass graph (same graph on all 8 cores)."""
    R = B * S  # total rows
    SHARD = R // N_CORES  # output row-shard per core
    KT = S // 128  # k-position tiles per batch
    QW = S // 2  # q-half width (one attention work unit)
    NQC = max(1, QW // 512)
    QC = QW // NQC  # matmul N-chunk
    RCW = min(512, S)  # qp/kp chunk width
    RC = R // RCW
    RC_B = RC // B  # chunks per batch
    KT_B = KT

    nc = bacc.Bacc("TRN2", target_bir_lowering=False, debug=False,
                   num_devices=N_CORES)

    qT = nc.dram_tensor("qT", [D, R], BF16, kind="ExternalInput")
    kT = nc.dram_tensor("kT", [D, R], BF16, kind="ExternalInput")
    vT = nc.dram_tensor("vT", [D, R], BF16, kind="ExternalInput")
    wqT = nc.dram_tensor("wqT", [D, HD], BF16, kind="ExternalInput")
    wkT = nc.dram_tensor("wkT", [D, HD], BF16, kind="ExternalInput")
    wvT = nc.dram_tensor("wvT", [D, HD], BF16, kind="ExternalInput")
    woT = nc.dram_tensor("woT", [D, D], BF16, kind="ExternalInput")
    q_res = nc.dram_tensor("q_res", [SHARD, D], F32, kind="ExternalInput")
    gamma = nc.dram_tensor("gamma", [1, D], MMDT, kind="ExternalInput")
    beta = nc.dram_tensor("beta", [1, D], MMDT, kind="ExternalInput")

    attn_out = nc.dram_tensor("attn_out", [B * H_PER_CORE, S, S], BF16,
                              kind="ExternalOutput")
    x_out = nc.dram_tensor("x_out", [SHARD, D], F32, kind="ExternalOutput")

    from contextlib import ExitStack

    with tile.TileContext(nc) as tc:
        with (
            tc.tile_pool(name="persist", bufs=1) as pp,
            tc.tile_pool(name="dram", bufs=1, space="DRAM") as dp,
        ):
            qp = [pp.tile([128, RCW], BF16, tag=f"qp{rc}", name=f"qp{rc}")
                  for rc in range(RC)]
            kp = [pp.tile([128, RCW], BF16, tag=f"kp{rc}", name=f"kp{rc}")
                  for rc in range(RC)]
            v_sb = {(h, b): pp.tile([128, KT_B * 96], BF16,
                                    tag=f"v{h}{b}", name=f"v_sb{h}{b}")
                    for h in range(H_PER_CORE) for b in range(B)}
            ctxn = [pp.tile([64, R], BF16, tag=f"ctxn{h}", name=f"ctxn{h}")
                    for h in range(H_PER_CORE)]
            gamma_rep = pp.tile([128, D], F32, tag="gamma_rep")
            beta_rep = pp.tile([128, D], F32, tag="beta_rep")
            ones_sb = pp.tile([128, 128], MMDT, tag="ones_sb")
            n_rt = (SHARD + 127) // 128
            wq_sb = pp.tile([128, D], BF16, tag="wq")
            wk_sb = pp.tile([128, D], BF16, tag="wk")
            wv_sb = pp.tile([128, D], BF16, tag="wv")

            a2a_in = dp.tile([N_CORES * HD, SHARD], BF16, tag="a2a_in")
            a2a_out = dp.tile([N_CORES * HD, SHARD], BF16, tag="a2a_out")

            # ---- phase 0: constants + weight/residual prefetch ----
            for t in v_sb.values():
                nc.vector.memset(t[:, :], 1.0)
            nc.vector.memset(ones_sb[:, :].bitcast(F32), 1.0)
            gb_sb = pp.tile([1, D], MMDT, tag="gb")
            bb_sb = pp.tile([1, D], MMDT, tag="bb")
            nc.sync.dma_start(out=gb_sb[:, :], in_=gamma[:, :])
            nc.sync.dma_start(out=bb_sb[:, :], in_=beta[:, :])
            for wsb, wdr in ((wq_sb, wqT), (wk_sb, wkT), (wv_sb, wvT)):
                nc.sync.dma_start(
                    out=wsb[:, :].rearrange("p (t m) -> p t m", t=I_TILES),
                    in_=wdr[:, :].rearrange("(t p) m -> p t m", p=128))
            stk = ExitStack()
            p1 = stk.enter_context(tc.tile_pool(name="p1", bufs=1))
            p2a = stk.enter_context(tc.tile_pool(name="p2a", bufs=2))
            p2w = stk.enter_context(tc.tile_pool(name="p2w", bufs=3))
            p2r = stk.enter_context(tc.tile_pool(name="p2r", bufs=2))
            ps_1 = stk.enter_context(
                tc.tile_pool(name="ps_1", bufs=2, space="PSUM"))
            ps_s = stk.enter_context(
                tc.tile_pool(name="ps_s", bufs=2, space="PSUM"))
            ps_c = stk.enter_context(
                tc.tile_pool(name="ps_c", bufs=1, space="PSUM"))
            if True:
                for rep, row in ((gamma_rep, gb_sb), (beta_rep, bb_sb)):
                    psb0 = ps_c.tile([128, D], F32, tag="psc",
                                     name=f"psb0_{rep.name}")
                    for jc in range(D // 512):
                        js = slice(512 * jc, 512 * (jc + 1))
                        nc.tensor.matmul(psb0[:, js], ones_sb[0:1, :],
                                         row[0:1, js], start=True, stop=True)
                    nc.vector.tensor_copy(rep[:, :], psb0[:, :])

            def phase1(b):
                """Project q/k/v for batch b's rows."""
                bufs = 1
                for rcl in range(RC_B):
                    rc = b * RC_B + rcl
                    rs = slice(RCW * rc, RCW * (rc + 1))
                    qt_b = p1.tile([128, I_TILES * RCW], BF16,
                                   tag=f"qt{b}", bufs=bufs,
                                   name=f"qt{b}_{rcl}")
                    kt_b = p1.tile([128, I_TILES * RCW], BF16,
                                   tag=f"kt{b}", bufs=bufs,
                                   name=f"kt{b}_{rcl}")
                    vt_b = p1.tile([128, I_TILES * RCW], BF16,
                                   tag=f"vt{b}", bufs=bufs,
                                   name=f"vt{b}_{rcl}")
                    for sb_t, dr_t in ((qt_b, qT), (kt_b, kT),
                                       (vt_b, vT)):
                        nc.sync.dma_start(
                            out=sb_t[:, :].rearrange("p (t r) -> p t r",
                                                     t=I_TILES),
                            in_=dr_t[:, rs].rearrange("(t p) r -> p t r",
                                                      p=128))
                    psq = ps_1.tile([128, RCW], F32, tag="p1s",
                                    name=f"psq{rc}")
                    psk = ps_1.tile([128, RCW], F32, tag="p1s",
                                    name=f"psk{rc}")
                    for it in range(I_TILES):
                        ic = slice(128 * it, 128 * (it + 1))
                        ir = slice(RCW * it, RCW * (it + 1))
                        first, last = it == 0, it == I_TILES - 1
                        nc.tensor.matmul(psq[:, :], wq_sb[:, ic],
                                         qt_b[:, ir], start=first, stop=last)
                        nc.tensor.matmul(psk[:, :], wk_sb[:, ic],
                                         kt_b[:, ir], start=first, stop=last)
                    nc.vector.tensor_copy(qp[rc][:, :], psq[:, :])
                    nc.vector.tensor_copy(kp[rc][:, :], psk[:, :])
                    for t in range(RCW // 128):
                        psv = ps_1.tile([128, 128], F32, tag="p1s",
                                        name=f"psv{rc}_{t}")
                        for it in range(I_TILES):
                            ic = slice(128 * it, 128 * (it + 1))
                            nc.tensor.matmul(
                                psv[:, :],
                                vt_b[:, RCW * it + 128 * t:
                                     RCW * it + 128 * (t + 1)],
                                wv_sb[:, ic],
                                start=(it == 0), stop=(it == I_TILES - 1))
                        r_tile = rc * (RCW // 128) + t
                        b_ix, kt_ix = divmod(r_tile, KT_B)
                        for h in range(H_PER_CORE):
                            nc.vector.tensor_copy(
                                v_sb[(h, b_ix)][:, 96 * kt_ix:96 * kt_ix + 64],
                                psv[:, 64 * h:64 * (h + 1)])

            def pkslice(col0, width):
                t = col0 // RCW
                o = col0 - t * RCW
                assert o + width <= RCW
                return t, slice(o, o + width)

            def phase2(b):
                """Attention for batch b."""
                for h in range(H_PER_CORE):
                    hs = slice(64 * h, 64 * (h + 1))
                    for qh in range(2):
                        q0 = S * b + QW * qh
                        psc = ps_c.tile([128, QW], F32, tag="psc",
                                        name=f"psc{b}{h}{qh}")
                        pss_of = {}

                        def scores(kt_i, _q0=q0, _hs=hs, _b=b, _h=h, _qh=qh):
                            kt_t, kt_s = pkslice(S * _b + 128 * kt_i, 128)
                            ps = ps_s.tile([128, QW], F32, tag="pss",
                                           name=f"pss{_b}{_h}{_qh}_{kt_i}")
                            for qc in range(NQC):
                                qt_t, qt_s = pkslice(_q0 + QC * qc, QC)
                                nc.tensor.matmul(
                                    ps[:, QC * qc:QC * (qc + 1)],
                                    kp[kt_t][_hs, kt_s],
                                    qp[qt_t][_hs, qt_s],
                                    start=True, stop=True)
                            pss_of[kt_i] = ps

                        scores(0)
                        at_tiles = []
                        for kt_i in range(KT):
                            at = p2a.tile([128, QW], BF16,
                                          tag=f"attn{kt_i}",
                                          name=f"at{b}{h}{qh}_{kt_i}")
                            at_tiles.append(at)
                            nc.scalar.activation(at[:, :],
                                                 pss_of.pop(kt_i)[:, :],
                                                 AF.Exp, scale=0.125)
                            if kt_i + 1 < KT:
                                scores(kt_i + 1)
                            for qc in range(NQC):
                                cs = slice(QC * qc, QC * (qc + 1))
                                nc.tensor.matmul(
                                    psc[0:96, cs],
                                    v_sb[(h, b)][:, 96 * kt_i:96 * kt_i + 96],
                                    at[:, cs],
                                    start=(kt_i == 0), stop=(kt_i == KT - 1))
                        # denominator -> replicated reciprocal
                        den_sb = p2r.tile([128, QW], MMDT, tag="den_sb",
                                          bufs=1)
                        recip = p2r.tile([128, QW], F32, tag="recip")
                        recip_b = p2r.tile([128, QW], BF16, tag="recip_b")
                        nc.vector.tensor_copy(den_sb[64:65, :],
                                              psc[64:65, :])
                        psb = ps_s.tile([128, QW], F32, tag="pss",
                                        name=f"psb{b}{h}{qh}")
                        for qc in range(NQC):
                            cs = slice(QC * qc, QC * (qc + 1))
                            nc.tensor.matmul(psb[:, cs], ones_sb[64:65, :],
                                             den_sb[64:65, cs],
                                             start=True, stop=True)
                        nc.vector.reciprocal_approx_fast(recip[:, :],
                                                         psb[:, :])
                        nc.vector.tensor_copy(recip_b[:, :], recip[:, :])
                        nc.vector.tensor_mul(
                            ctxn[h][0:64, q0:q0 + QW],
                            psc[0:64, :], recip[0:64, :])
                        plane = b * H_PER_CORE + h
                        for kt_i in range(KT):
                            wt = p2w.tile([128, QW], BF16, tag="wt")
                            nc.vector.tensor_mul(wt[:, :],
                                                 at_tiles[kt_i][:, :],
                                                 recip_b[:, :])
                            nc.gpsimd.dma_start(
                                out=attn_out[plane,
                                             128 * kt_i:128 * (kt_i + 1),
                                             QW * qh:QW * (qh + 1)],
                                in_=wt[:, :])

            for b in range(B):
                phase1(b)
                phase2(b)
            stk.close()

            # ---- phase 3: all-to-all of the context ----
            for j in range(N_CORES):
                ss = slice(SHARD * j, SHARD * (j + 1))
                for h in range(H_PER_CORE):
                    nc.sync.dma_start(
                        out=a2a_in[HD * j + 64 * h:HD * j + 64 * (h + 1), :],
                        in_=ctxn[h][0:64, ss])
            nc.gpsimd.collective_compute(
                "AllToAll",
                ALU.bypass,
                replica_groups=[list(range(N_CORES))],
                ins=[a2a_in[:, :].opt()],
                outs=[a2a_out[:, :].opt()],
            )

            # ---- phase 4: W_O projection + residual + LayerNorm ----
            with (
                tc.tile_pool(name="p4", bufs=1) as p4,
                tc.tile_pool(name="p4x", bufs=2) as p4x,
                tc.tile_pool(name="ps_o", bufs=2, space="PSUM") as ps_o,
            ):
                wo_sb = p4.tile([128, I_TILES * D], BF16, tag="wo")
                qres_sb = p4.tile([128, n_rt * D], F32, tag="qres")
                for t in range(I_TILES):
                    nc.sync.dma_start(out=wo_sb[:, D * t:D * (t + 1)],
                                      in_=woT[128 * t:128 * (t + 1), :])
                for t in range(n_rt):
                    rp0 = min(128, SHARD - 128 * t)
                    nc.sync.dma_start(
                        out=qres_sb[:rp0, D * t:D * (t + 1)],
                        in_=q_res[128 * t:128 * t + rp0, :])
                ctxf = p4.tile([128, I_TILES * SHARD], BF16, tag="ctxf")
                for t in range(I_TILES):
                    nc.sync.dma_start(
                        out=ctxf[:, SHARD * t:SHARD * (t + 1)],
                        in_=a2a_out[128 * t:128 * (t + 1), :])

                xcs, rstds, rps = [], [], []
                _P4VAR = []
                for rt in range(n_rt):
                    rp = min(128, SHARD - 128 * rt)
                    rps.append(rp)
                    xsb = p4x.tile([128, D], F32, tag="xsb",
                                   name=f"xsb{rt}", bufs=n_rt)
                    for jc in range(2):
                        js = slice(512 * jc, 512 * (jc + 1))
                        pso = ps_o.tile([128, 512], F32, tag="pso")
                        for t in range(I_TILES):
                            nc.tensor.matmul(
                                pso[:rp, :],
                                ctxf[:, SHARD * t + 128 * rt:
                                     SHARD * t + 128 * rt + rp],
                                wo_sb[:, D * t + 512 * jc:
                                      D * t + 512 * (jc + 1)],
                                start=(t == 0), stop=(t == I_TILES - 1))
                        nc.vector.tensor_add(
                            xsb[:rp, js], pso[:rp, :],
                            qres_sb[:rp, D * rt + 512 * jc:
                                    D * rt + 512 * (jc + 1)])
                    mu = p4x.tile([128, 1], F32, tag="mu")
                    var = p4x.tile([128, 1], F32, tag="var",
                                   name=f"var{rt}", bufs=n_rt)
                    xc = p4x.tile([128, D], F32, tag="xc",
                                  name=f"xc{rt}", bufs=n_rt)
                    sq = p4x.tile([128, D], F32, tag="sq")
                    nc.vector.tensor_reduce(mu[:rp, :], xsb[:rp, :],
                                            axis=mybir.AxisListType.X,
                                            op=ALU.add)
                    nc.vector.tensor_scalar_mul(mu[:rp, :], mu[:rp, :],
                                                1.0 / D)
                    nc.vector.tensor_scalar(xc[:rp, :], xsb[:rp, :],
                                            mu[:rp, :], None,
                                            op0=ALU.subtract)
                    nc.vector.scalar_tensor_tensor(
                        sq[:rp, :], in0=xc[:rp, :], scalar=1.0,
                        in1=xc[:rp, :], op0=ALU.mult, op1=ALU.mult,
                        accum_out=var[:rp, :])
                    nc.vector.tensor_scalar(var[:rp, :], var[:rp, :],
                                            1.0 / D, 1e-5,
                                            op0=ALU.mult, op1=ALU.add)
                    xcs.append(xc)
                    _P4VAR.append(var)
                # rstd = exp(-0.5*ln(var+eps)); batch per activation set so
                # the ACT table loads once per function, not per row-tile
                for rt in range(n_rt):
                    rstd = p4x.tile([128, 1], F32, tag="rstd",
                                    name=f"rstd{rt}", bufs=n_rt)
                    nc.scalar.activation(rstd[:rps[rt], :],
                                         _P4VAR[rt][:rps[rt], :], AF.Ln)
                    rstds.append(rstd)
                for rt in range(n_rt):
                    nc.scalar.activation(rstds[rt][:rps[rt], :],
                                         rstds[rt][:rps[rt], :], AF.Exp,
                                         scale=-0.5)
                for rt in range(n_rt):
                    rp = rps[rt]
                    rsl = slice(128 * rt, 128 * rt + rp)
                    xo = p4x.tile([128, D], F32, tag="xo")
                    nc.vector.scalar_tensor_tensor(
                        xo[:rp, :], in0=xcs[rt][:rp, :],
                        scalar=rstds[rt][:rp, :],
                        in1=gamma_rep[:rp, :], op0=ALU.mult, op1=ALU.mult)
                    nc.vector.tensor_add(xo[:rp, :], xo[:rp, :],
                                         beta_rep[:rp, :])
                    nc.sync.dma_start(out=x_out[rsl, :], in_=xo[:rp, :])

    nc.compile()
    return nc


_NC_CACHE = {}


def _get_nc(S):
    if S not in _NC_CACHE:
        _NC_CACHE[S] = build_nc(S)
    return _NC_CACHE[S]


def make_in_maps(Q, K, V, W_Q, W_K, W_V, W_O, ln_gamma, ln_beta):
    S = Q.shape[1]
    R = B * S
    SHARD = R // N_CORES
    bf16 = mybir.dt.np(BF16)
    Q2 = np.asarray(Q, np.float32).reshape(R, D)
    QT = np.ascontiguousarray(Q2.T)
    KT_ = np.ascontiguousarray(np.asarray(K, np.float32).reshape(R, D).T)
    VT_ = np.ascontiguousarray(np.asarray(V, np.float32).reshape(R, D).T)
    woT = np.ascontiguousarray(np.asarray(W_O, np.float32).T).astype(bf16)
    QTb = QT.astype(bf16)
    KTb = KT_.astype(bf16)
    VTb = VT_.astype(bf16)
    g = np.asarray(ln_gamma, np.float32).reshape(1, D)
    bta = np.asarray(ln_beta, np.float32).reshape(1, D)
    in_maps = []
    for c in range(N_CORES):
        hsl = slice(HD * c, HD * (c + 1))
        in_maps.append({
            "qT": QTb, "kT": KTb, "vT": VTb,
            "wqT": np.ascontiguousarray(
                np.asarray(W_Q, np.float32)[hsl, :].T).astype(bf16),
            "wkT": np.ascontiguousarray(
                np.asarray(W_K, np.float32)[hsl, :].T).astype(bf16),
            "wvT": np.ascontiguousarray(
                np.asarray(W_V, np.float32)[hsl, :].T).astype(bf16),
            "woT": woT,
            "q_res": np.ascontiguousarray(Q2[SHARD * c:SHARD * (c + 1), :]),
            "gamma": g, "beta": bta,
        })
    return in_maps


def assemble(results, S):
    R = B * S
    SHARD = R // N_CORES
    x = np.empty((R, D), np.float32)
    attn = np.empty((B, N_HEAD, S, S), np.float32)
    for c in range(N_CORES):
        x[SHARD * c:SHARD * (c + 1), :] = results[c]["x_out"]
        a = np.asarray(results[c]["attn_out"], dtype=np.float32)
        for b in range(B):
            for h in range(H_PER_CORE):
                attn[b, H_PER_CORE * c + h] = a[b * H_PER_CORE + h].T
    return x.reshape(B, S, D), attn


def kernel(Q, K, V, W_Q, W_K, W_V, W_O, ln_gamma, ln_beta):
    from concourse.bass_utils import run_bass_kernel_spmd

    S = Q.shape[1]
    nc = _get_nc(S)
    in_maps = make_in_maps(Q, K, V, W_Q, W_K, W_V, W_O, ln_gamma, ln_beta)
    res = run_bass_kernel_spmd(nc, in_maps, core_ids=list(range(N_CORES)))
    return assemble(res.results, S)


# revision 24
# speedup vs baseline: 1.5685x; 1.0443x over previous
"""Multi-head attention + residual + LayerNorm on 8 Trainium2 NeuronCores.

Problem: nn_MultiHeadAttention_446676599424
  B=2, S=2048, D_MODEL=1024, N_HEAD=16, D_K=64
  reference returns (x, attn_weights):
    x [B, S, D]  = LayerNorm(attn_out @ W_O.T + Q)
    attn_weights [B, H, S, S] = softmax(q k^T / sqrt(d_k))

Sharding: tensor-parallel over heads — 2 heads per core. Each core:
  1. projects q/k/v for its 2 heads (inputs pre-transposed on host so the
     contraction dim lands on SBUF partitions); q/k kept in float32r for
     score precision, v cast to bf16,
  2. computes scores TRANSPOSED  s_T[k, q] = k_proj q_proj^T (so the
     softmax denominator falls out of the attn@v matmul as a ones-column
     of the stationary operand and attn@v needs no transposes),
  3. exp via ScalarE (scale 1/8 fused) into bf16 tiles, denominator =
     row 64 of the context matmul output, replicated across partitions
     with a K=1 PE matmul, reciprocal on DVE,
  4. writes attn in [b, h, k, q] bf16 (host transposes/upcasts),
  5. AllToAll redistributes the per-head context [d, r] to row shards,
  6. every core computes its 512-row shard of context @ W_O.T + residual
     + LayerNorm (fp32) and returns it; host concatenates.
"""

import sys

if "/opt/trn_rl_repo" not in sys.path:
    sys.path.insert(0, "/opt/trn_rl_repo")

import numpy as np

import concourse.bacc as bacc
import concourse.mybir as mybir
from concourse import tile

F32 = mybir.dt.float32
BF16 = mybir.dt.bfloat16
AF = mybir.ActivationFunctionType
ALU = mybir.AluOpType

B = 2
D = 1024
N_HEAD = 16
D_K = 64
N_CORES = 8
H_PER_CORE = N_HEAD // N_CORES  # 2
HD = H_PER_CORE * D_K  # 128, per-core head-dim block
I_TILES = D // 128  # 8 contraction tiles for the projections

# float32r: fp32 storage, full-rate PE with reduced multiply mantissa.
MMDT = mybir.dt.float32r


def build_nc(S=2048):
    """Build the per-core Bass graph (same graph on all 8 cores)."""
    R = B * S  # total rows
    SHARD = R // N_CORES  # output row-shard per core
    KT = S // 128  # k-position tiles per batch
    QW = S // 2  # q-half width (one attention work unit)
    NQC = max(1, QW // 512)
    QC = QW // NQC  # matmul N-chunk
    RCW = min(512, S)  # qp/kp chunk width
    RC = R // RCW
    RC_B = RC // B  # chunks per batch
    KT_B = KT

    nc = bacc.Bacc("TRN2", target_bir_lowering=False, debug=False,
                   num_devices=N_CORES)

    qT = nc.dram_tensor("qT", [D, R], BF16, kind="ExternalInput")
    kT = nc.dram_tensor("kT", [D, R], BF16, kind="ExternalInput")
    vT = nc.dram_tensor("vT", [D, R], BF16, kind="ExternalInput")
    wqT = nc.dram_tensor("wqT", [D, HD], BF16, kind="ExternalInput")
    wkT = nc.dram_tensor("wkT", [D, HD], BF16, kind="ExternalInput")
    wvT = nc.dram_tensor("wvT", [D, HD], BF16, kind="ExternalInput")
    woT = nc.dram_tensor("woT", [D, D], BF16, kind="ExternalInput")
    q_res = nc.dram_tensor("q_res", [SHARD, D], F32, kind="ExternalInput")
    gamma = nc.dram_tensor("gamma", [1, D], MMDT, kind="ExternalInput")
    beta = nc.dram_tensor("beta", [1, D], MMDT, kind="ExternalInput")

    attn_out = nc.dram_tensor("attn_out", [B * H_PER_CORE, S, S], BF16,
                              kind="ExternalOutput")
    x_out = nc.dram_tensor("x_out", [SHARD, D], F32, kind="ExternalOutput")

    from contextlib import ExitStack

    with tile.TileContext(nc) as tc:
        with (
            tc.tile_pool(name="persist", bufs=1) as pp,
            tc.tile_pool(name="dram", bufs=1, space="DRAM") as dp,
        ):
            qp = [pp.tile([128, RCW], BF16, tag=f"qp{rc}", name=f"qp{rc}")
                  for rc in range(RC)]
            kp = [pp.tile([128, RCW], BF16, tag=f"kp{rc}", name=f"kp{rc}")
                  for rc in range(RC)]
            v_sb = {(h, b): pp.tile([128, KT_B * 96], BF16,
                                    tag=f"v{h}{b}", name=f"v_sb{h}{b}")
                    for h in range(H_PER_CORE) for b in range(B)}
            ctxn = [pp.tile([64, R], BF16, tag=f"ctxn{h}", name=f"ctxn{h}")
                    for h in range(H_PER_CORE)]
            gamma_rep = pp.tile([128, D], F32, tag="gamma_rep")
            beta_rep = pp.tile([128, D], F32, tag="beta_rep")
            ones_sb = pp.tile([128, 128], MMDT, tag="ones_sb")
            n_rt = (SHARD + 127) // 128
            wq_sb = pp.tile([128, D], BF16, tag="wq")
            wk_sb = pp.tile([128, D], BF16, tag="wk")
            wv_sb = pp.tile([128, D], BF16, tag="wv")

            a2a_in = dp.tile([N_CORES * HD, SHARD], BF16, tag="a2a_in")
            a2a_out = dp.tile([N_CORES * HD, SHARD], BF16, tag="a2a_out")

            # ---- phase 0: constants + weight/residual prefetch ----
            for t in v_sb.values():
                nc.vector.memset(t[:, :], 1.0)
            nc.vector.memset(ones_sb[:, :].bitcast(F32), 1.0)
            gb_sb = pp.tile([1, D], MMDT, tag="gb")
            bb_sb = pp.tile([1, D], MMDT, tag="bb")
            nc.sync.dma_start(out=gb_sb[:, :], in_=gamma[:, :])
            nc.sync.dma_start(out=bb_sb[:, :], in_=beta[:, :])
            for wsb, wdr in ((wq_sb, wqT), (wk_sb, wkT), (wv_sb, wvT)):
                nc.sync.dma_start(
                    out=wsb[:, :].rearrange("p (t m) -> p t m", t=I_TILES),
                    in_=wdr[:, :].rearrange("(t p) m -> p t m", p=128))
            stk = ExitStack()
            p1 = stk.enter_context(tc.tile_pool(name="p1", bufs=1))
            p2a = stk.enter_context(tc.tile_pool(name="p2a", bufs=2))
            p2w = stk.enter_context(tc.tile_pool(name="p2w", bufs=3))
            p2r = stk.enter_context(tc.tile_pool(name="p2r", bufs=2))
            ps_1 = stk.enter_context(
                tc.tile_pool(name="ps_1", bufs=2, space="PSUM"))
            ps_s = stk.enter_context(
                tc.tile_pool(name="ps_s", bufs=2, space="PSUM"))
            ps_c = stk.enter_context(
                tc.tile_pool(name="ps_c", bufs=1, space="PSUM"))
            if True:
                for rep, row in ((gamma_rep, gb_sb), (beta_rep, bb_sb)):
                    psb0 = ps_c.tile([128, D], F32, tag="psc",
                                     name=f"psb0_{rep.name}")
                    for jc in range(D // 512):
                        js = slice(512 * jc, 512 * (jc + 1))
                        nc.tensor.matmul(psb0[:, js], ones_sb[0:1, :],
                                         row[0:1, js], start=True, stop=True)
                    nc.vector.tensor_copy(rep[:, :], psb0[:, :])

            def phase1_rc(b, rcl):
                """Project q/k/v for one r-chunk of batch b."""
                bufs = 1
                if True:
                    rc = b * RC_B + rcl
                    rs = slice(RCW * rc, RCW * (rc + 1))
                    qt_b = p1.tile([128, I_TILES * RCW], BF16,
                                   tag=f"qt{b}", bufs=bufs,
                                   name=f"qt{b}_{rcl}")
                    kt_b = p1.tile([128, I_TILES * RCW], BF16,
                                   tag=f"kt{b}", bufs=bufs,
                                   name=f"kt{b}_{rcl}")
                    vt_b = p1.tile([128, I_TILES * RCW], BF16,
                                   tag=f"vt{b}", bufs=bufs,
                                   name=f"vt{b}_{rcl}")
                    for sb_t, dr_t in ((qt_b, qT), (kt_b, kT),
                                       (vt_b, vT)):
                        nc.sync.dma_start(
                            out=sb_t[:, :].rearrange("p (t r) -> p t r",
                                                     t=I_TILES),
                            in_=dr_t[:, rs].rearrange("(t p) r -> p t r",
                                                      p=128))
                    psq = ps_1.tile([128, RCW], F32, tag="p1s",
                                    name=f"psq{rc}")
                    psk = ps_1.tile([128, RCW], F32, tag="p1s",
                                    name=f"psk{rc}")
                    for it in range(I_TILES):
                        ic = slice(128 * it, 128 * (it + 1))
                        ir = slice(RCW * it, RCW * (it + 1))
                        first, last = it == 0, it == I_TILES - 1
                        nc.tensor.matmul(psq[:, :], wq_sb[:, ic],
                                         qt_b[:, ir], start=first, stop=last)
                        nc.tensor.matmul(psk[:, :], wk_sb[:, ic],
                                         kt_b[:, ir], start=first, stop=last)
                    nc.vector.tensor_copy(qp[rc][:, :], psq[:, :])
                    nc.vector.tensor_copy(kp[rc][:, :], psk[:, :])
                    for t in range(RCW // 128):
                        psv = ps_1.tile([128, 128], F32, tag="p1s",
                                        name=f"psv{rc}_{t}")
                        for it in range(I_TILES):
                            ic = slice(128 * it, 128 * (it + 1))
                            nc.tensor.matmul(
                                psv[:, :],
                                vt_b[:, RCW * it + 128 * t:
                                     RCW * it + 128 * (t + 1)],
                                wv_sb[:, ic],
                                start=(it == 0), stop=(it == I_TILES - 1))
                        r_tile = rc * (RCW // 128) + t
                        b_ix, kt_ix = divmod(r_tile, KT_B)
                        for h in range(H_PER_CORE):
                            nc.vector.tensor_copy(
                                v_sb[(h, b_ix)][:, 96 * kt_ix:96 * kt_ix + 64],
                                psv[:, 64 * h:64 * (h + 1)])

            def pkslice(col0, width):
                t = col0 // RCW
                o = col0 - t * RCW
                assert o + width <= RCW
                return t, slice(o, o + width)

            pending = []  # deferred (wt-normalize, dma) jobs

            def emit_pending(n):
                for _ in range(min(n, len(pending))):
                    pending.pop(0)()

            def phase2_unit(b, h, qh):
                """Attention for one (batch, head, q-half) unit."""
                if True:
                    hs = slice(64 * h, 64 * (h + 1))
                    if True:
                        q0 = S * b + QW * qh
                        psc = ps_c.tile([128, QW], F32, tag="psc",
                                        name=f"psc{b}{h}{qh}")
                        pss_of = {}

                        def scores(kt_i, _q0=q0, _hs=hs, _b=b, _h=h, _qh=qh):
                            kt_t, kt_s = pkslice(S * _b + 128 * kt_i, 128)
                            ps = ps_s.tile([128, QW], F32, tag="pss",
                                           name=f"pss{_b}{_h}{_qh}_{kt_i}")
                            for qc in range(NQC):
                                qt_t, qt_s = pkslice(_q0 + QC * qc, QC)
                                nc.tensor.matmul(
                                    ps[:, QC * qc:QC * (qc + 1)],
                                    kp[kt_t][_hs, kt_s],
                                    qp[qt_t][_hs, qt_s],
                                    start=True, stop=True)
                            pss_of[kt_i] = ps

                        scores(0)
                        at_tiles = []
                        for kt_i in range(KT):
                            at = p2a.tile([128, QW], BF16,
                                          tag=f"attn{kt_i}",
                                          name=f"at{b}{h}{qh}_{kt_i}")
                            at_tiles.append(at)
                            nc.scalar.activation(at[:, :],
                                                 pss_of.pop(kt_i)[:, :],
                                                 AF.Exp, scale=0.125)
                            if kt_i + 1 < KT:
                                scores(kt_i + 1)
                            for qc in range(NQC):
                                cs = slice(QC * qc, QC * (qc + 1))
                                nc.tensor.matmul(
                                    psc[0:96, cs],
                                    v_sb[(h, b)][:, 96 * kt_i:96 * kt_i + 96],
                                    at[:, cs],
                                    start=(kt_i == 0), stop=(kt_i == KT - 1))
                            emit_pending(1)
                        # denominator -> replicated reciprocal
                        den_sb = p2r.tile([128, QW], MMDT, tag="den_sb",
                                          bufs=1)
                        recip = p2r.tile([128, QW], F32, tag="recip")
                        recip_b = p2r.tile([128, QW], BF16, tag="recip_b")
                        nc.vector.tensor_copy(den_sb[64:65, :],
                                              psc[64:65, :])
                        psb = ps_s.tile([128, QW], F32, tag="pss",
                                        name=f"psb{b}{h}{qh}")
                        for qc in range(NQC):
                            cs = slice(QC * qc, QC * (qc + 1))
                            nc.tensor.matmul(psb[:, cs], ones_sb[64:65, :],
                                             den_sb[64:65, cs],
                                             start=True, stop=True)
                        nc.vector.reciprocal_approx_fast(recip[:, :],
                                                         psb[:, :])
                        nc.vector.tensor_copy(recip_b[:, :], recip[:, :])
                        nc.vector.tensor_mul(
                            ctxn[h][0:64, q0:q0 + QW],
                            psc[0:64, :], recip[0:64, :])
                        plane = b * H_PER_CORE + h

                        def wt_job(kt_i, _at=at_tiles, _rb=recip_b,
                                   _plane=plane, _qh=qh, _b=b, _h=h):
                            wt = p2w.tile([128, QW], BF16, tag="wt",
                                          name=f"wt{_b}{_h}{_qh}_{kt_i}")
                            nc.vector.tensor_mul(wt[:, :],
                                                 _at[kt_i][:, :],
                                                 _rb[:, :])
                            nc.gpsimd.dma_start(
                                out=attn_out[_plane,
                                             128 * kt_i:128 * (kt_i + 1),
                                             QW * _qh:QW * (_qh + 1)],
                                in_=wt[:, :])

                        for kt_i in range(KT):
                            pending.append(
                                lambda k=kt_i, f=wt_job: f(k))

            for rcl in range(RC_B):
                phase1_rc(0, rcl)
            units0 = [(0, h, qh) for h in range(H_PER_CORE)
                      for qh in range(2)]
            units1 = [(1, h, qh) for h in range(H_PER_CORE)
                      for qh in range(2)]
            for i, (b_, h_, qh_) in enumerate(units0):
                phase2_unit(b_, h_, qh_)
                if i < RC_B:
                    phase1_rc(1, i)
            for b_, h_, qh_ in units1:
                phase2_unit(b_, h_, qh_)
            emit_pending(len(pending))
            stk.close()

            # ---- phase 3: all-to-all of the context ----
            for j in range(N_CORES):
                ss = slice(SHARD * j, SHARD * (j + 1))
                for h in range(H_PER_CORE):
                    nc.sync.dma_start(
                        out=a2a_in[HD * j + 64 * h:HD * j + 64 * (h + 1), :],
                        in_=ctxn[h][0:64, ss])
            nc.gpsimd.collective_compute(
                "AllToAll",
                ALU.bypass,
                replica_groups=[list(range(N_CORES))],
                ins=[a2a_in[:, :].opt()],
                outs=[a2a_out[:, :].opt()],
            )

            # ---- phase 4: W_O projection + residual + LayerNorm ----
            with (
                tc.tile_pool(name="p4", bufs=1) as p4,
                tc.tile_pool(name="p4x", bufs=2) as p4x,
                tc.tile_pool(name="ps_o", bufs=2, space="PSUM") as ps_o,
            ):
                wo_sb = p4.tile([128, I_TILES * D], BF16, tag="wo")
                qres_sb = p4.tile([128, n_rt * D], F32, tag="qres")
                for t in range(I_TILES):
                    nc.sync.dma_start(out=wo_sb[:, D * t:D * (t + 1)],
                                      in_=woT[128 * t:128 * (t + 1), :])
                for t in range(n_rt):
                    rp0 = min(128, SHARD - 128 * t)
                    nc.sync.dma_start(
                        out=qres_sb[:rp0, D * t:D * (t + 1)],
                        in_=q_res[128 * t:128 * t + rp0, :])
                ctxf = p4.tile([128, I_TILES * SHARD], BF16, tag="ctxf")
                for t in range(I_TILES):
                    nc.sync.dma_start(
                        out=ctxf[:, SHARD * t:SHARD * (t + 1)],
                        in_=a2a_out[128 * t:128 * (t + 1), :])

                xcs, rstds, rps = [], [], []
                _P4VAR = []
                for rt in range(n_rt):
                    rp = min(128, SHARD - 128 * rt)
                    rps.append(rp)
                    xsb = p4x.tile([128, D], F32, tag="xsb",
                                   name=f"xsb{rt}", bufs=n_rt)
                    for jc in range(2):
                        js = slice(512 * jc, 512 * (jc + 1))
                        pso = ps_o.tile([128, 512], F32, tag="pso")
                        for t in range(I_TILES):
                            nc.tensor.matmul(
                                pso[:rp, :],
                                ctxf[:, SHARD * t + 128 * rt:
                                     SHARD * t + 128 * rt + rp],
                                wo_sb[:, D * t + 512 * jc:
                                      D * t + 512 * (jc + 1)],
                                start=(t == 0), stop=(t == I_TILES - 1))
                        nc.vector.tensor_add(
                            xsb[:rp, js], pso[:rp, :],
                            qres_sb[:rp, D * rt + 512 * jc:
                                    D * rt + 512 * (jc + 1)])
                    mu = p4x.tile([128, 1], F32, tag="mu")
                    var = p4x.tile([128, 1], F32, tag="var",
                                   name=f"var{rt}", bufs=n_rt)
                    xc = p4x.tile([128, D], F32, tag="xc",
                                  name=f"xc{rt}", bufs=n_rt)
                    sq = p4x.tile([128, D], F32, tag="sq")
                    nc.vector.tensor_reduce(mu[:rp, :], xsb[:rp, :],
                                            axis=mybir.AxisListType.X,
                                            op=ALU.add)
                    nc.vector.tensor_scalar_mul(mu[:rp, :], mu[:rp, :],
                                                1.0 / D)
                    nc.vector.tensor_scalar(xc[:rp, :], xsb[:rp, :],
                                            mu[:rp, :], None,
                                            op0=ALU.subtract)
                    nc.vector.scalar_tensor_tensor(
                        sq[:rp, :], in0=xc[:rp, :], scalar=1.0,
                        in1=xc[:rp, :], op0=ALU.mult, op1=ALU.mult,
                        accum_out=var[:rp, :])
                    nc.vector.tensor_scalar(var[:rp, :], var[:rp, :],
                                            1.0 / D, 1e-5,
                                            op0=ALU.mult, op1=ALU.add)
                    xcs.append(xc)
                    _P4VAR.append(var)
                # rstd = exp(-0.5*ln(var+eps)); batch per activation set so
                # the ACT table loads once per function, not per row-tile
                for rt in range(n_rt):
                    rstd = p4x.tile([128, 1], F32, tag="rstd",
                                    name=f"rstd{rt}", bufs=n_rt)
                    nc.scalar.activation(rstd[:rps[rt], :],
                                         _P4VAR[rt][:rps[rt], :], AF.Ln)
                    rstds.append(rstd)
                for rt in range(n_rt):
                    nc.scalar.activation(rstds[rt][:rps[rt], :],
                                         rstds[rt][:rps[rt], :], AF.Exp,
                                         scale=-0.5)
                for rt in range(n_rt):
                    rp = rps[rt]
                    rsl = slice(128 * rt, 128 * rt + rp)
                    xo = p4x.tile([128, D], F32, tag="xo")
                    nc.vector.scalar_tensor_tensor(
                        xo[:rp, :], in0=xcs[rt][:rp, :],
                        scalar=rstds[rt][:rp, :],
                        in1=gamma_rep[:rp, :], op0=ALU.mult, op1=ALU.mult)
                    nc.vector.tensor_add(xo[:rp, :], xo[:rp, :],
                                         beta_rep[:rp, :])
                    nc.sync.dma_start(out=x_out[rsl, :], in_=xo[:rp, :])

    nc.compile()
    return nc


_NC_CACHE = {}


def _get_nc(S):
    if S not in _NC_CACHE:
        _NC_CACHE[S] = build_nc(S)
    return _NC_CACHE[S]


def make_in_maps(Q, K, V, W_Q, W_K, W_V, W_O, ln_gamma, ln_beta):
    S = Q.shape[1]
    R = B * S
    SHARD = R // N_CORES
    bf16 = mybir.dt.np(BF16)
    Q2 = np.asarray(Q, np.float32).reshape(R, D)
    QT = np.ascontiguousarray(Q2.T)
    KT_ = np.ascontiguousarray(np.asarray(K, np.float32).reshape(R, D).T)
    VT_ = np.ascontiguousarray(np.asarray(V, np.float32).reshape(R, D).T)
    woT = np.ascontiguousarray(np.asarray(W_O, np.float32).T).astype(bf16)
    QTb = QT.astype(bf16)
    KTb = KT_.astype(bf16)
    VTb = VT_.astype(bf16)
    g = np.asarray(ln_gamma, np.float32).reshape(1, D)
    bta = np.asarray(ln_beta, np.float32).reshape(1, D)
    in_maps = []
    for c in range(N_CORES):
        hsl = slice(HD * c, HD * (c + 1))
        in_maps.append({
            "qT": QTb, "kT": KTb, "vT": VTb,
            "wqT": np.ascontiguousarray(
                np.asarray(W_Q, np.float32)[hsl, :].T).astype(bf16),
            "wkT": np.ascontiguousarray(
                np.asarray(W_K, np.float32)[hsl, :].T).astype(bf16),
            "wvT": np.ascontiguousarray(
                np.asarray(W_V, np.float32)[hsl, :].T).astype(bf16),
            "woT": woT,
            "q_res": np.ascontiguousarray(Q2[SHARD * c:SHARD * (c + 1), :]),
            "gamma": g, "beta": bta,
        })
    return in_maps


def assemble(results, S):
    R = B * S
    SHARD = R // N_CORES
    x = np.empty((R, D), np.float32)
    attn = np.empty((B, N_HEAD, S, S), np.float32)
    for c in range(N_CORES):
        x[SHARD * c:SHARD * (c + 1), :] = results[c]["x_out"]
        a = np.asarray(results[c]["attn_out"], dtype=np.float32)
        for b in range(B):
            for h in range(H_PER_CORE):
                attn[b, H_PER_CORE * c + h] = a[b * H_PER_CORE + h].T
    return x.reshape(B, S, D), attn


def kernel(Q, K, V, W_Q, W_K, W_V, W_O, ln_gamma, ln_beta):
    from concourse.bass_utils import run_bass_kernel_spmd

    S = Q.shape[1]
    nc = _get_nc(S)
    in_maps = make_in_maps(Q, K, V, W_Q, W_K, W_V, W_O, ln_gamma, ln_beta)
    res = run_bass_kernel_spmd(nc, in_maps, core_ids=list(range(N_CORES)))
    return assemble(res.results, S)
